# revision 1
# baseline (speedup 1.0000x reference)
"""Trainium2 Bass kernel for nn_CombinatorialClassifier (segment_reduce).

Strategy (8 NeuronCores, tensor-parallel over the num_partitionings axis):
  - Core i owns partitionings {2i, 2i+1}: a [2000, 2048] slice of W.
  - On device: logits = x @ Wshard.T (+ b row folded into the matmul),
    per-partitioning softmax -> probs [64, 2000], then a gpsimd ap_gather
    picks probs[b, idx(p, c)] for every class and the two partitionings are
    summed -> per-core partial [64, 50000].
  - Host: sum the 8 partials over cores (the all-reduce of the sharding
    hint), normalize over classes, log.

Gather layout: the probs tile is duplicated across SBUF partitions 0-63 /
64-127 so all 8 gpsimd Q7 cores work; quadrant A handles classes
[0, 25000), quadrant B [25000, 50000).
"""

import os
from contextlib import ExitStack

import numpy as np

import concourse.bacc as bacc
import concourse.mybir as mybir
import concourse.tile as tile
from concourse import bass_utils

B, P, K, C, D = 64, 16, 1000, 50000, 2048
ESP = 1e-20
NCORES = 8
PPC = P // NCORES        # partitionings per core
NLOC = PPC * K           # local logits width (2000)
NT = 500                 # matmul N-tile (PSUM bank: 500 fp32 <= 512)
NNT = NLOC // NT         # 4 N-tiles
DCH = D // 128           # 16 contraction chunks of 128
CQ = C // 2              # classes per gather quadrant stream (25000)
JC = 2048                # classes per gather call per quadrant

_chunks = []
_c = 0
while _c < CQ:
    _chunks.append(min(JC, CQ - _c))
    _c += JC
IDXCOLS = sum(2 * jc // 16 for jc in _chunks)  # int16 cols of gather indices

_F32 = mybir.dt.float32
_F16 = mybir.dt.float16
_I16 = mybir.dt.int16

_CACHE = {}
LAST_RESULTS = None


def _build_nc():
    nc = bacc.Bacc(
        "TRN2",
        target_bir_lowering=False,
        debug=False,
        enable_asserts=False,
        num_devices=NCORES,
    )
    xT_d = nc.dram_tensor("xT", [D, B], _F16, kind="ExternalInput")
    wtb_d = nc.dram_tensor("wtb", [D + 1, NLOC], _F16, kind="ExternalInput")
    idx_d = nc.dram_tensor("idx", [128, IDXCOLS], _I16, kind="ExternalInput")
    out_d = nc.dram_tensor("part_out", [B, C], _F32, kind="ExternalOutput")

    with tile.TileContext(nc) as tc, ExitStack() as ctx:
        const = ctx.enter_context(tc.tile_pool(name="const", bufs=1))
        wpool = ctx.enter_context(tc.tile_pool(name="w", bufs=3))
        spool = ctx.enter_context(tc.tile_pool(name="stats", bufs=1))
        gpool = ctx.enter_context(tc.tile_pool(name="g", bufs=3))
        apool = ctx.enter_context(tc.tile_pool(name="a", bufs=3))
        psum = ctx.enter_context(
            tc.tile_pool(name="psum", bufs=1, space="PSUM")
        )

        xt = const.tile([128, DCH, B], _F16)
        nc.sync.dma_start(xt[:], xT_d.ap().rearrange("(c p) b -> p c b", p=128))
        ones = const.tile([1, B], _F16)
        nc.vector.memset(ones[:], 1.0)
        bias = const.tile([1, NLOC], _F16)
        nc.sync.dma_start(bias[:], wtb_d[D : D + 1, :])
        idx_sb = const.tile([128, IDXCOLS], _I16)
        nc.sync.dma_start(idx_sb[:], idx_d.ap())
        gsrc = const.tile([128, NLOC], _F32)

        # ---- logits = x @ Wshard.T + b, accumulated in 4 PSUM banks ----
        ps = [psum.tile([B, NT], _F32, tag=f"ps{n}", name=f"ps{n}") for n in range(NNT)]
        for j in range(DCH):
            wt = wpool.tile([128, NLOC], _F16)
            nc.sync.dma_start(wt[:], wtb_d[128 * j : 128 * (j + 1), :])
            for n in range(NNT):
                nc.tensor.matmul(
                    ps[n][:],
                    xt[:, j, :],
                    wt[:, NT * n : NT * (n + 1)],
                    start=(j == 0),
                    stop=False,
                )
        for n in range(NNT):
            nc.tensor.matmul(
                ps[n][:],
                ones[:],
                bias[:, NT * n : NT * (n + 1)],
                start=False,
                stop=True,
            )

        # ---- per-partitioning softmax -> probs in gsrc[0:64] ----
        mx = spool.tile([B, NNT], _F32)
        for n in range(NNT):
            nc.vector.reduce_max(
                mx[:, n : n + 1], ps[n][:], axis=mybir.AxisListType.X
            )
        neg = spool.tile([B, PPC], _F32)
        for h in range(PPC):
            nc.vector.tensor_tensor(
                neg[:, h : h + 1],
                mx[:, 2 * h : 2 * h + 1],
                mx[:, 2 * h + 1 : 2 * h + 2],
                op=mybir.AluOpType.max,
            )
        nc.vector.tensor_scalar_mul(neg[:], neg[:], -1.0)
        sacc = spool.tile([B, NNT], _F32)
        for n in range(NNT):
            h = n // 2
            nc.scalar.activation(
                gsrc[0:B, NT * n : NT * (n + 1)],
                ps[n][:],
                mybir.ActivationFunctionType.Exp,
                bias=neg[:, h : h + 1],
                accum_out=sacc[:, n : n + 1],
            )
        rec = spool.tile([B, PPC], _F32)
        for h in range(PPC):
            nc.vector.tensor_tensor(
                rec[:, h : h + 1],
                sacc[:, 2 * h : 2 * h + 1],
                sacc[:, 2 * h + 1 : 2 * h + 2],
                op=mybir.AluOpType.add,
            )
        nc.vector.reciprocal(rec[:], rec[:])
        for h in range(PPC):
            nc.vector.tensor_scalar_mul(
                gsrc[0:B, K * h : K * (h + 1)],
                gsrc[0:B, K * h : K * (h + 1)],
                rec[:, h : h + 1],
            )
        # duplicate probs for the second gpsimd quadrant
        nc.sync.dma_start(gsrc[B : 2 * B, :], gsrc[0:B, :])

        # ---- gather + partitioning-sum + store ----
        offs = 0
        cum = 0
        for jc in _chunks:
            S = 2 * jc // 16
            g = gpool.tile([128, 2 * JC], _F32, tag="g")
            nc.gpsimd.ap_gather(
                g[:, 0 : 2 * jc],
                gsrc[:, :],
                idx_sb[:, offs : offs + S],
                channels=128,
                num_elems=NLOC,
                d=1,
                num_idxs=2 * jc,
            )
            a = apool.tile([128, JC], _F32, tag="a")
            nc.vector.tensor_add(a[:, 0:jc], g[:, 0:jc], g[:, jc : 2 * jc])
            nc.sync.dma_start(out_d[:, cum : cum + jc], a[0:B, 0:jc])
            nc.sync.dma_start(
                out_d[:, CQ + cum : CQ + cum + jc], a[B : 2 * B, 0:jc]
            )
            offs += S
            cum += jc

    nc.compile()
    return nc


def _host_inputs(x, W, b, part):
    """Per-core input maps: xT, wtb (W.T shard + bias row), gather indices."""
    xT = np.ascontiguousarray(x.T.astype(np.float16))
    part = np.asarray(part).astype(np.int64, copy=False)
    in_maps = []
    for i in range(NCORES):
        r0 = NLOC * i
        wtb = np.empty((D + 1, NLOC), np.float16)
        wtb[:D] = W[r0 : r0 + NLOC].T
        wtb[D] = b[r0 : r0 + NLOC]

        pa = part[2 * i] - (2 * i) * K            # [C] in [0, K)
        pb = part[2 * i + 1] - (2 * i + 1) * K + K  # [C] in [K, 2K)
        idxh = np.empty((128, IDXCOLS), np.int16)
        off = 0
        c0 = 0
        for jc in _chunks:
            S = 2 * jc // 16
            for q in range(2):
                base = q * CQ + c0
                L = np.concatenate(
                    [pa[base : base + jc], pb[base : base + jc]]
                ).astype(np.int16)
                blk = L.reshape(S, 16).T  # out col i <- (partition i%16, col i//16)
                for g4 in range(4):
                    p0 = q * 64 + g4 * 16
                    idxh[p0 : p0 + 16, off : off + S] = blk
            off += S
            c0 += jc
        in_maps.append({"xT": xT, "wtb": wtb, "idx": idxh})
    return in_maps


def kernel(**inputs):
    global LAST_RESULTS
    x = np.asarray(inputs["input"], dtype=np.float32)
    W = np.asarray(inputs["W"], dtype=np.float32)
    b = np.asarray(inputs["b"], dtype=np.float32)
    part = np.asarray(inputs["partitionings"])
    assert x.shape == (B, D) and W.shape == (P * K, D)

    if "nc" not in _CACHE:
        _CACHE["nc"] = _build_nc()
    nc = _CACHE["nc"]

    in_maps = _host_inputs(x, W, b, part)
    trace = bool(int(os.environ.get("BASSK_TRACE", "0")))
    res = bass_utils.run_bass_kernel_spmd(
        nc,
        in_maps,
        core_ids=list(range(NCORES)),
        trace=trace,
        tmpdir=os.environ.get("BASSK_TRACE_DIR") or None,
    )
    LAST_RESULTS = res

    acc = res.results[0]["part_out"].astype(np.float32)
    for i in range(1, NCORES):
        acc = acc + res.results[i]["part_out"]
    tot = acc.sum(axis=1, keepdims=True)
    return np.log(acc / tot + ESP).astype(np.float32)

